# revision 36
# baseline (speedup 1.0000x reference)
"""Multi-head attention Trainium2 kernel, 8-core batch+head sharded.

Sharding: cores 0-3 -> batch 0, cores 4-7 -> batch 1; each core computes 4
heads. Host compacts queries by q_mask and keys by v_mask (masked softmax
over the kept key subset equals the reference's additive-mask softmax),
transposes/packs inputs, and sums the 4 per-core partial output projections
per batch (the row-sharded-Wo "all-reduce"), adds bo, scatters rows back.

v7 vs the original kernel: softmax denominator fused into AV (ones column
appended to each head's V -> z lands in row 64 of the [65, W] accumulator,
eliminating the Z matmul stream), real-query trim (QR=max(nq), not
128-padded), K/Q projections stream t-outer behind per-d-tile DMAs, the
first head-pair pass runs scores+exp only under the xv DMA window with its
AV and the V projection interleaved into the second pass, normalize /
output-projection work is scheduled as fillers inside later kt loops (PE
never drains -> stays at full p-state), one fast reciprocal per head pair
(partition-parallel z), and Scalar does only exp plus half the po casts.

Self-contained: hardcodes B=2,S=2048,D=1024,H=16,HS=64,OUT=1024.
"""
import sys, types

sys.path.insert(0, '/opt/trn_rl_repo')

# ---- NTFF profile hook (image's antenv lacks axon_hooks) ----
if "antenv.axon_hooks" not in sys.modules:
    _hook_mod = types.ModuleType("antenv.axon_hooks")
    _hook_mod._hook = None
    def _set_hook(h, _m=_hook_mod):
        _m._hook = h
    def _get_hook(_m=_hook_mod):
        return _m._hook
    _hook_mod.set_axon_ntff_profile_hook = _set_hook
    _hook_mod.get_axon_ntff_profile_hook = _get_hook
    sys.modules["antenv.axon_hooks"] = _hook_mod
    try:
        from trn_agent_boot.trn_boot import _ntff_profile_via_ctypes
        _set_hook(_ntff_profile_via_ctypes('/opt/axon/libaxon_pjrt.so'))
    except Exception:
        pass

import numpy as np
import ml_dtypes
import concourse.bass as bass
import concourse.tile as tile
import concourse.mybir as mybir
from concourse import bass_utils, bacc

B, S, D, H, HS, OUT = 2, 2048, 1024, 16, 64, 1024
HPC = 4          # heads per core
NCORES = 8
DT = D // 128    # 8 d-tiles
F32 = mybir.dt.float32
F16 = mybir.dt.float16
SCALE = float(1.0 / np.sqrt(HS))
KPAD_BIAS = -1e5  # exp underflows to exactly 0.0


def _blocks(total, maxw=512):
    """Near-equal widths <= maxw (avoids degenerate tail blocks)."""
    nb = (total + maxw - 1) // maxw
    base, rem = divmod(total, nb)
    return [base + (1 if i < rem else 0) for i in range(nb)]


def build_kernel(QR, SKP):
    """One SPMD Bass program. QR: real (shared max) query count; SKP:
    padded (mult of 128) key count."""
    SKT = SKP // 128
    QRP = ((QR + 31) // 32) * 32   # 64B-aligned DMA rows
    qblk = _blocks(QR)
    kblk = _blocks(SKP)
    nc = bacc.Bacc("TRN2", target_bir_lowering=False, debug=False,
                   num_devices=NCORES)

    xq_d = nc.dram_tensor('xq', [DT, 128, QRP], F16, kind='ExternalInput').ap()
    xk_d = nc.dram_tensor('xk', [DT, 128, SKP], F16, kind='ExternalInput').ap()
    xv_d = nc.dram_tensor('xv', [DT, 128, SKP], F16, kind='ExternalInput').ap()
    wq_d = nc.dram_tensor('wq', [128, DT, 256], F16, kind='ExternalInput').ap()
    wk_d = nc.dram_tensor('wk', [128, DT, 256], F16, kind='ExternalInput').ap()
    wv_d = nc.dram_tensor('wv', [128, DT, 256], F16, kind='ExternalInput').ap()
    wo_d = nc.dram_tensor('wo', [2, 128, OUT], F16, kind='ExternalInput').ap()
    qkb_d = nc.dram_tensor('qkb', [128, 4], F32, kind='ExternalInput').ap()
    vb_d = nc.dram_tensor('vb', [1, 256], F32, kind='ExternalInput').ap()
    kbias_d = nc.dram_tensor('kbias', [128, SKT], F32, kind='ExternalInput').ap()
    outp = nc.dram_tensor('outp', [QR, OUT], F16, kind='ExternalOutput').ap()

    with tile.TileContext(nc) as tc, \
         nc.allow_low_precision(reason="fp16 attention within tolerance"):
        with tc.tile_pool(name="const", bufs=1) as constp, \
             tc.tile_pool(name="persist", bufs=1) as persist, \
             tc.tile_pool(name="etile", bufs=4) as etile, \
             tc.tile_pool(name="e0tile", bufs=1) as e0tile, \
             tc.tile_pool(name="work", bufs=3) as work:

            # ---- constants ----
            wq_sb = constp.tile([128, DT, 256], F16)
            wk_sb = constp.tile([128, DT, 256], F16)
            wv_sb = constp.tile([128, DT, 256], F16)
            wo_sb = constp.tile([128, 2, OUT], F16)
            qkb_sb = constp.tile([128, 4], F32)
            vb_bc = constp.tile([128, 256], F32)
            kbias_sb = constp.tile([128, SKT], F32)
            ones_h = constp.tile([128, 64], F16)
            ones_f = constp.tile([128, 64], F32)
            nc.vector.memset(ones_f, 1.0)
            nc.vector.tensor_copy(ones_h, ones_f)
            # pre-load the ScalarE exp table during stage-A DMA
            warm = constp.tile([128, 1], F32)
            nc.scalar.activation(warm, ones_f[:, 0:1],
                                 mybir.ActivationFunctionType.Exp)

            # ---- persistent activations ----
            xk_sb = persist.tile([128, DT, SKP], F16)
            xv_sb = persist.tile([128, DT, SKP], F16)
            xq_sb = persist.tile([128, DT, QRP], F16)
            qt_sb = persist.tile([128, 2, QR], F16)    # [:, pair, :]: Q^T
            kt_sb = persist.tile([128, 2, SKP], F16)
            # V natural + ones col per head: [:, skt, h*80:h*80+64] = V_h,
            # col h*80+64 = 1.0 (softmax denominator comes free in AV)
            v_sb = persist.tile([128, SKT, 4 * 80], F16)
            ot_sb = persist.tile([128, 2, QR], F16)    # normalized O^T

            vones = bass.AP(tensor=v_sb.tensor, offset=v_sb.offset + 64,
                            ap=[v_sb.ap[0], [4 * 80, SKT], [80, 4], [1, 1]])
            vones_in = bass.AP(tensor=ones_f.tensor, offset=ones_f.offset,
                               ap=[ones_f.ap[0], [4, SKT], [1, 4], [1, 1]])
            nc.vector.tensor_copy(vones, vones_in)

            # ---- input DMAs, ordered by first use; tiny const DMAs go
            # after the x tensors (each dma_start costs ~0.75us of Sync
            # dispatch, so the critical-path ones come first) ----
            nc.sync.dma_start(out=wk_sb, in_=wk_d)
            for t in range(DT):
                nc.sync.dma_start(out=xk_sb[:, t, :], in_=xk_d[t])
            nc.sync.dma_start(out=wq_sb, in_=wq_d)
            for t in range(DT):
                nc.sync.dma_start(out=xq_sb[:, t, :], in_=xq_d[t])
            nc.sync.dma_start(out=qkb_sb, in_=qkb_d)
            nc.sync.dma_start(out=kbias_sb, in_=kbias_d)
            nc.sync.dma_start(out=wv_sb, in_=wv_d)
            xv_src = bass.AP(tensor=xv_d.tensor, offset=xv_d.offset,
                             ap=[[SKP, 128], [128 * SKP, DT], [1, SKP]])
            nc.sync.dma_start(out=xv_sb, in_=xv_src)
            nc.sync.dma_start(out=vb_bc, in_=bass.AP(
                tensor=vb_d.tensor, offset=vb_d.offset,
                ap=[[0, 128], vb_d.ap[1]]))
            nc.sync.dma_start(out=wo_sb[:, 0, :], in_=wo_d[0])
            nc.sync.dma_start(out=wo_sb[:, 1, :], in_=wo_d[1])

            # ---- stage A: first K chunk + full Q projection, t-outer so
            # matmuls stream behind the per-d-tile DMAs; later K chunks are
            # emitted as fillers inside the first attention pass ----
            with tc.tile_pool(name="psA", bufs=3, space="PSUM") as psA:
                for w_sb, x_sb, pt_sb, blks, bcol0 in (
                        (wk_sb, xk_sb, kt_sb, kblk[:1], 2),
                        (wq_sb, xq_sb, qt_sb, qblk, 0)):
                    pp = [psA.tile([128, 2, 512], F32, tag="s",
                                   name=f"pp{ci}") for ci in range(len(blks))]
                    for t in range(DT):
                        b0 = 0
                        for ci, blen in enumerate(blks):
                            for p in range(2):
                                nc.tensor.matmul(
                                    pp[ci][:, p, :blen],
                                    w_sb[:, t, p * 128:(p + 1) * 128],
                                    x_sb[:, t, b0:b0 + blen],
                                    start=(t == 0), stop=(t == DT - 1))
                            b0 += blen
                    b0 = 0
                    for ci, blen in enumerate(blks):
                        for p in range(2):
                            nc.vector.tensor_scalar_add(
                                pt_sb[:, p, b0:b0 + blen], pp[ci][:, p, :blen],
                                qkb_sb[:, bcol0 + p: bcol0 + p + 1])
                        b0 += blen

            # ---- stage B/C ----
            with tc.tile_pool(name="psS", bufs=2, space="PSUM") as psS, \
                 tc.tile_pool(name="psO", bufs=2, space="PSUM") as psO:

                def vproj(skt):
                    pv = psS.tile([128, 2, 512], F32, tag="s", name="pv")
                    for t in range(DT):
                        nc.tensor.matmul(
                            pv[:, 0, :256],
                            xv_sb[:, t, skt * 128:(skt + 1) * 128],
                            wv_sb[:, t, :],
                            start=(t == 0), stop=(t == DT - 1))
                    vout = bass.AP(tensor=v_sb.tensor,
                                   offset=v_sb.offset + skt * 4 * 80,
                                   ap=[v_sb.ap[0], [80, 4], [1, 64]])
                    pv4 = bass.AP(tensor=pv.tensor, offset=pv.offset,
                                  ap=[pv.ap[0], [64, 4], [1, 64]])
                    vb4 = bass.AP(tensor=vb_bc.tensor, offset=vb_bc.offset,
                                  ap=[vb_bc.ap[0], [64, 4], [1, 64]])
                    nc.vector.tensor_add(vout, pv4, vb4)

                def scores_exp(bq0, W, pair, kt, epool, ebufs):
                    st = psS.tile([128, 2, 512], F32, tag="s")
                    for hh in range(2):
                        nc.tensor.matmul(
                            st[:, hh, :W],
                            kt_sb[hh * 64:(hh + 1) * 64, pair,
                                  kt * 128:(kt + 1) * 128],
                            qt_sb[hh * 64:(hh + 1) * 64, pair, bq0:bq0 + W],
                            start=True, stop=True)
                    e_t = epool.tile([128, 2, 512], F16, tag="e",
                                     bufs=ebufs)
                    nc.scalar.activation(
                        e_t[:, :, :W], st[:, :, :W],
                        mybir.ActivationFunctionType.Exp,
                        bias=kbias_sb[:, kt:kt + 1], scale=SCALE)
                    return e_t

                def av(op_t, pair, kt, e_t, W):
                    for hh in range(2):
                        h = pair * 2 + hh
                        nc.tensor.matmul(
                            op_t[:65, hh, :W],
                            v_sb[:, kt, h * 80:h * 80 + 65],
                            e_t[:, hh, :W],
                            start=(kt == 0), stop=(kt == SKT - 1))

                def emit_pass(bq0, W, pair, op_t, fillers):
                    for kt in range(SKT):
                        e_t = scores_exp(bq0, W, pair, kt, etile, 4)
                        if kt < len(fillers) and fillers[kt] is not None:
                            fillers[kt]()
                        av(op_t, pair, kt, e_t, W)
                    for f in fillers[SKT:]:
                        if f is not None:
                            f()

                def emit_norm(bq0, W, pair, op_t):
                    # one fast reciprocal covers both heads' z rows
                    zinv = work.tile([128, 2, 512], F32, tag="zinv")
                    nc.vector.reciprocal_approx_fast(
                        zinv[:, :, :W], op_t[:, :, :W])
                    zinv16 = work.tile([128, 2, 512], F16, tag="zinv16")
                    nc.vector.tensor_copy(zinv16[64:65, :, :W],
                                          zinv[64:65, :, :W])
                    for hh in range(2):
                        zbc = psS.tile([64, 512], F32, tag="s",
                                       name=f"zbc{hh}")
                        nc.tensor.matmul(
                            zbc[:, :W],
                            ones_h[64:65, 0:64],
                            zinv16[64:65, hh, :W],
                            start=True, stop=True,
                            tile_position=(64, 0))
                        zbc_sb = work.tile([64, 512], F32, tag="zbc",
                                           name=f"zbcs{hh}")
                        nc.vector.tensor_copy(zbc_sb[:, :W], zbc[:, :W])
                        nc.vector.tensor_mul(
                            ot_sb[hh * 64:(hh + 1) * 64, pair,
                                  bq0:bq0 + W],
                            op_t[0:64, hh, :W], zbc_sb[:, :W])

                def outproj_unit(q0, qw, on_scalar):
                    def emit():
                        po = psS.tile([128, 2, 512], F32, tag="s", name="po")
                        for ch in range(2):
                            for kt in range(2):
                                nc.tensor.matmul(
                                    po[:qw, ch, :],
                                    ot_sb[:, kt, q0:q0 + qw],
                                    wo_sb[:, kt, ch * 512:(ch + 1) * 512],
                                    start=(kt == 0), stop=(kt == 1))
                        ob = work.tile([128, OUT], F16, tag="ob")
                        pof = bass.AP(tensor=po.tensor, offset=po.offset,
                                      ap=[[po.ap[0][0], qw], [1, OUT]])
                        if on_scalar:
                            nc.scalar.copy(ob[:qw, :], pof)
                        else:
                            nc.vector.tensor_copy(ob[:qw, :], pof)
                        nc.sync.dma_start(out=outp[q0:q0 + qw, :],
                                          in_=ob[:qw, :])
                    return emit

                def outproj_units(bq0, W):
                    units = []
                    q0 = bq0
                    while q0 < bq0 + W:
                        qw = min(128, bq0 + W - q0)
                        units.append(outproj_unit(q0, qw, len(units) % 2))
                        q0 += qw
                    return units

                # late K-proj chunks: per-d-tile pieces accumulating into a
                # psO buffer (psO is otherwise idle until the first AV)
                def kproj_pieces(kpp, b0, blen):
                    pieces = []
                    for t in range(DT):
                        def piece(t=t):
                            for p in range(2):
                                nc.tensor.matmul(
                                    kpp[:, p, :blen],
                                    wk_sb[:, t, p * 128:(p + 1) * 128],
                                    xk_sb[:, t, b0:b0 + blen],
                                    start=(t == 0), stop=(t == DT - 1))
                        pieces.append(piece)
                    def bias():
                        for p in range(2):
                            nc.vector.tensor_scalar_add(
                                kt_sb[:, p, b0:b0 + blen], kpp[:, p, :blen],
                                qkb_sb[:, 2 + p:3 + p])
                    pieces.append(bias)
                    return pieces

                # --- pass (0,0): scores+exp only; its AV runs in pass (0,1)
                # together with the streaming V projection. Late K chunks
                # fill the idle PE here, 3 pieces per kt so chunk ci lands
                # before the first score tile that reads it ---
                W0 = qblk[0]
                e0 = []
                kfill = []
                koff = kblk[0]
                for ci, blen in enumerate(kblk[1:]):
                    kpp = psO.tile([128, 2, 512], F32, tag="acc",
                                   name=f"kpp{ci}")
                    kfill.extend(kproj_pieces(kpp, koff, blen))
                    koff += blen
                for kt in range(SKT):
                    e0.append(scores_exp(0, W0, 0, kt, e0tile, SKT))
                    for f in kfill[3 * kt:3 * kt + 3]:
                        f()
                for f in kfill[3 * SKT:]:
                    f()

                op00 = psO.tile([128, 2, 512], F32, tag="acc", name="op00")
                op01 = psO.tile([128, 2, 512], F32, tag="acc", name="op01")
                vproj(0)
                if SKT > 1:
                    vproj(1)

                def pass01_filler(kt):
                    def emit():
                        if kt + 2 < SKT:
                            vproj(kt + 2)
                        av(op00, 0, kt, e0[kt], W0)
                    return emit

                fillers = [pass01_filler(kt) for kt in range(SKT)]
                fillers.append(lambda: emit_norm(0, W0, 0, op00))
                emit_pass(0, W0, 1, op01, fillers)

                pend_norm = (0, W0, 1, op01)
                pend_out = (0, W0)
                bq0 = W0
                for bi in range(1, len(qblk)):
                    W = qblk[bi]
                    for pair in range(2):
                        fillers = [None]
                        if pend_norm is not None:
                            fillers.append(
                                (lambda a: lambda: emit_norm(*a))(pend_norm))
                            pend_norm = None
                        if pair == 0 and pend_out is not None:
                            fillers.extend([None, None])
                            fillers.extend(outproj_units(*pend_out))
                            pend_out = None
                        op_t = psO.tile([128, 2, 512], F32, tag="acc")
                        emit_pass(bq0, W, pair, op_t, fillers)
                        pend_norm = (bq0, W, pair, op_t)
                    pend_out = (bq0, W)
                    bq0 += W
                emit_norm(*pend_norm)
                for u in outproj_units(*pend_out):
                    u()

    nc.compile()
    return nc


_NC_CACHE = {}


def _get_kernel(QR, SKP):
    key = (QR, SKP)
    if key not in _NC_CACHE:
        _NC_CACHE[key] = build_kernel(QR, SKP)
    return _NC_CACHE[key]


def _ref_numpy(q, k, v, Wq, bq, Wk, bk, Wv, bv, Wo, bo, qm, vm):
    """Exact-reference fallback for degenerate masks (all-zero v_mask)."""
    qp = (q @ Wq + bq).reshape(S, H, HS)
    kp = (k @ Wk + bk).reshape(S, H, HS)
    vp = (v @ Wv + bv).reshape(S, H, HS)
    a = np.einsum('qhd,khd->hqk', qp, kp) / np.sqrt(HS)
    a = a - (1.0 - vm[None, None, :]) * 1e12
    a = a - a.max(-1, keepdims=True)
    e = np.exp(a)
    p = e / e.sum(-1, keepdims=True)
    o = np.einsum('hqk,khd->qhd', p, vp).reshape(S, H * HS)
    return (o @ Wo + bo) * qm[:, None]


def run(query, key, value, Wq, bq, Wk, bk, Wv, bv, Wo, bo, q_mask, v_mask,
        trace=False):
    query = np.asarray(query, np.float32)
    key = np.asarray(key, np.float32)
    value = np.asarray(value, np.float32)
    Wq, bq = np.asarray(Wq, np.float32), np.asarray(bq, np.float32)
    Wk, bk = np.asarray(Wk, np.float32), np.asarray(bk, np.float32)
    Wv, bv = np.asarray(Wv, np.float32), np.asarray(bv, np.float32)
    Wo, bo = np.asarray(Wo, np.float32), np.asarray(bo, np.float32)
    q_mask = np.asarray(q_mask)
    v_mask = np.asarray(v_mask)

    qidx = [np.nonzero(q_mask[b])[0] for b in range(B)]
    kidx = [np.nonzero(v_mask[b])[0] for b in range(B)]
    host_fallback = [len(kidx[b]) == 0 for b in range(B)]

    QR = max([128] + [len(i) for b, i in enumerate(qidx) if not host_fallback[b]])
    # small leftover beyond a 512 multiple goes to the host so the device
    # runs uniform 512-wide blocks
    if QR > 512 and QR % 512 <= 64:
        QR = 512 * (QR // 512)
    nk = max([128] + [len(i) for b, i in enumerate(kidx) if not host_fallback[b]])
    SKP = ((nk + 127) // 128) * 128
    SKT = SKP // 128
    QRP = ((QR + 31) // 32) * 32

    nc = _get_kernel(QR, SKP)

    in_maps = []
    for c in range(NCORES):
        b, hg = c // 4, c % 4
        hc = slice(hg * HPC * HS, (hg + 1) * HPC * HS)  # this core's 256 head cols
        xq = np.zeros((QRP, D), np.float32)
        xk = np.zeros((SKP, D), np.float32)
        xv = np.zeros((SKP, D), np.float32)
        if not host_fallback[b]:
            nq_dev = min(len(qidx[b]), QR)
            xq[:nq_dev] = query[b][qidx[b][:nq_dev]]
            xk[:len(kidx[b])] = key[b][kidx[b]]
            xv[:len(kidx[b])] = value[b][kidx[b]]
        qkb = np.stack([bq[hc][:128], bq[hc][128:],
                        bk[hc][:128], bk[hc][128:]], axis=1)
        nkb = len(kidx[b]) if not host_fallback[b] else 0
        kbias = np.where(np.arange(SKP) < nkb, 0.0, KPAD_BIAS).astype(np.float32)
        in_maps.append({
            'xq': np.ascontiguousarray(xq.T.reshape(DT, 128, QRP)).astype(np.float16),
            'xk': np.ascontiguousarray(xk.T.reshape(DT, 128, SKP)).astype(np.float16),
            'xv': np.ascontiguousarray(xv.T.reshape(DT, 128, SKP)).astype(np.float16),
            'wq': np.ascontiguousarray(Wq[:, hc].reshape(DT, 128, 256).transpose(1, 0, 2)).astype(np.float16),
            'wk': np.ascontiguousarray(Wk[:, hc].reshape(DT, 128, 256).transpose(1, 0, 2)).astype(np.float16),
            'wv': np.ascontiguousarray(Wv[:, hc].reshape(DT, 128, 256).transpose(1, 0, 2)).astype(np.float16),
            'wo': np.ascontiguousarray(Wo[hc, :].reshape(2, 128, OUT)).astype(np.float16),
            'qkb': np.ascontiguousarray(qkb),
            'vb': np.ascontiguousarray(bv[hc].reshape(1, 256)),
            'kbias': np.ascontiguousarray(kbias.reshape(SKT, 128).T),
        })

    res = bass_utils.run_bass_kernel_spmd(
        nc, in_maps, core_ids=list(range(NCORES)), trace=trace)

    out = np.zeros((B, S, OUT), np.float32)
    for b in range(B):
        if host_fallback[b]:
            out[b] = _ref_numpy(query[b], key[b], value[b], Wq, bq, Wk, bk,
                                Wv, bv, Wo, bo,
                                q_mask[b].astype(np.float32),
                                v_mask[b].astype(np.float32))
            continue
        acc = np.zeros((QR, OUT), np.float32)
        for c in range(4 * b, 4 * b + 4):
            acc += res.results[c]['outp'].astype(np.float32)
        nqb = len(qidx[b])
        nq_dev = min(nqb, QR)
        out[b][qidx[b][:nq_dev]] = acc[:nq_dev] + bo
        if nqb > nq_dev:
            # host-assist for the few query rows beyond the 512-multiple
            rows = qidx[b][nq_dev:]
            qp = (query[b][rows] @ Wq + bq).reshape(-1, H, HS)
            kp = (key[b][kidx[b]] @ Wk + bk).reshape(-1, H, HS)
            vp = (value[b][kidx[b]] @ Wv + bv).reshape(-1, H, HS)
            a = np.einsum('qhd,khd->hqk', qp, kp) / np.sqrt(HS)
            a = a - a.max(-1, keepdims=True)
            e = np.exp(a)
            p = e / e.sum(-1, keepdims=True)
            o = np.einsum('hqk,khd->qhd', p, vp).reshape(len(rows), H * HS)
            out[b][rows] = o @ Wo + bo
    return out, res


def kernel(**inputs):
    out, _ = run(**inputs)
    return out


# revision 37
# speedup vs baseline: 1.0208x; 1.0208x over previous
"""Multi-head attention Trainium2 kernel, 8-core batch+head sharded.

Sharding: cores 0-3 -> batch 0, cores 4-7 -> batch 1; each core computes 4
heads. Host compacts queries by q_mask and keys by v_mask (masked softmax
over the kept key subset equals the reference's additive-mask softmax),
transposes/packs inputs, and sums the 4 per-core partial output projections
per batch (the row-sharded-Wo "all-reduce"), adds bo, scatters rows back.

v7 vs the original kernel: softmax denominator fused into AV (ones column
appended to each head's V -> z lands in row 64 of the [65, W] accumulator,
eliminating the Z matmul stream), real-query trim (QR=max(nq), not
128-padded), K/Q projections stream t-outer behind per-d-tile DMAs, the
first head-pair pass runs scores+exp only under the xv DMA window with its
AV and the V projection interleaved into the second pass, normalize /
output-projection work is scheduled as fillers inside later kt loops (PE
never drains -> stays at full p-state), one fast reciprocal per head pair
(partition-parallel z), and Scalar does only exp plus half the po casts.

Self-contained: hardcodes B=2,S=2048,D=1024,H=16,HS=64,OUT=1024.
"""
import sys, types

sys.path.insert(0, '/opt/trn_rl_repo')

# ---- NTFF profile hook (image's antenv lacks axon_hooks) ----
if "antenv.axon_hooks" not in sys.modules:
    _hook_mod = types.ModuleType("antenv.axon_hooks")
    _hook_mod._hook = None
    def _set_hook(h, _m=_hook_mod):
        _m._hook = h
    def _get_hook(_m=_hook_mod):
        return _m._hook
    _hook_mod.set_axon_ntff_profile_hook = _set_hook
    _hook_mod.get_axon_ntff_profile_hook = _get_hook
    sys.modules["antenv.axon_hooks"] = _hook_mod
    try:
        from trn_agent_boot.trn_boot import _ntff_profile_via_ctypes
        _set_hook(_ntff_profile_via_ctypes('/opt/axon/libaxon_pjrt.so'))
    except Exception:
        pass

import numpy as np
import ml_dtypes
import concourse.bass as bass
import concourse.tile as tile
import concourse.mybir as mybir
from concourse import bass_utils, bacc

B, S, D, H, HS, OUT = 2, 2048, 1024, 16, 64, 1024
HPC = 4          # heads per core
NCORES = 8
DT = D // 128    # 8 d-tiles
F32 = mybir.dt.float32
F16 = mybir.dt.float16
SCALE = float(1.0 / np.sqrt(HS))
KPAD_BIAS = -1e5  # exp underflows to exactly 0.0


def _blocks(total, maxw=512):
    """Near-equal widths <= maxw (avoids degenerate tail blocks)."""
    nb = (total + maxw - 1) // maxw
    base, rem = divmod(total, nb)
    return [base + (1 if i < rem else 0) for i in range(nb)]


def build_kernel(QR, SKP):
    """One SPMD Bass program. QR: real (shared max) query count; SKP:
    padded (mult of 128) key count."""
    SKT = SKP // 128
    QRP = ((QR + 31) // 32) * 32   # 64B-aligned DMA rows
    qblk = _blocks(QR)
    kblk = _blocks(SKP)
    nc = bacc.Bacc("TRN2", target_bir_lowering=False, debug=False,
                   num_devices=NCORES)

    xq_d = nc.dram_tensor('xq', [DT, 128, QRP], F16, kind='ExternalInput').ap()
    xk_d = nc.dram_tensor('xk', [DT, 128, SKP], F16, kind='ExternalInput').ap()
    xv_d = nc.dram_tensor('xv', [DT, 128, SKP], F16, kind='ExternalInput').ap()
    wq_d = nc.dram_tensor('wq', [128, DT, 256], F16, kind='ExternalInput').ap()
    wk_d = nc.dram_tensor('wk', [128, DT, 256], F16, kind='ExternalInput').ap()
    wv_d = nc.dram_tensor('wv', [128, DT, 256], F16, kind='ExternalInput').ap()
    wo_d = nc.dram_tensor('wo', [2, 128, OUT], F16, kind='ExternalInput').ap()
    qkb_d = nc.dram_tensor('qkb', [128, 4], F32, kind='ExternalInput').ap()
    vb_d = nc.dram_tensor('vb', [1, 256], F32, kind='ExternalInput').ap()
    kbias_d = nc.dram_tensor('kbias', [128, SKT], F32, kind='ExternalInput').ap()
    outp = nc.dram_tensor('outp', [QR, OUT], F16, kind='ExternalOutput').ap()

    with tile.TileContext(nc) as tc, \
         nc.allow_low_precision(reason="fp16 attention within tolerance"):
        with tc.tile_pool(name="const", bufs=1) as constp, \
             tc.tile_pool(name="persist", bufs=1) as persist, \
             tc.tile_pool(name="etile", bufs=4) as etile, \
             tc.tile_pool(name="e0tile", bufs=1) as e0tile, \
             tc.tile_pool(name="work", bufs=3) as work:

            # ---- constants ----
            wq_sb = constp.tile([128, DT, 256], F16)
            wk_sb = constp.tile([128, DT, 256], F16)
            wv_sb = constp.tile([128, DT, 256], F16)
            wo_sb = constp.tile([128, 2, OUT], F16)
            qkb_sb = constp.tile([128, 4], F32)
            vb_bc = constp.tile([128, 256], F32)
            kbias_sb = constp.tile([128, SKT], F32)
            ones_h = constp.tile([128, 64], F16)
            ones_f = constp.tile([128, 64], F32)
            nc.vector.memset(ones_f, 1.0)
            nc.vector.tensor_copy(ones_h, ones_f)
            # pre-load the ScalarE exp table during stage-A DMA
            warm = constp.tile([128, 1], F32)
            nc.scalar.activation(warm, ones_f[:, 0:1],
                                 mybir.ActivationFunctionType.Exp)

            # ---- persistent activations ----
            xk_sb = persist.tile([128, DT, SKP], F16)
            xv_sb = persist.tile([128, DT, SKP], F16)
            xq_sb = persist.tile([128, DT, QRP], F16)
            qt_sb = persist.tile([128, 2, QR], F16)    # [:, pair, :]: Q^T
            kt_sb = persist.tile([128, 2, SKP], F16)
            # V natural + ones col per head: [:, skt, h*80:h*80+64] = V_h,
            # col h*80+64 = 1.0 (softmax denominator comes free in AV)
            v_sb = persist.tile([128, SKT, 4 * 80], F16)
            ot_sb = persist.tile([128, 2, QR], F16)    # normalized O^T

            vones = bass.AP(tensor=v_sb.tensor, offset=v_sb.offset + 64,
                            ap=[v_sb.ap[0], [4 * 80, SKT], [80, 4], [1, 1]])
            vones_in = bass.AP(tensor=ones_f.tensor, offset=ones_f.offset,
                               ap=[ones_f.ap[0], [4, SKT], [1, 4], [1, 1]])
            nc.vector.tensor_copy(vones, vones_in)

            # ---- input DMAs, ordered by first use; tiny const DMAs go
            # after the x tensors (each dma_start costs ~0.75us of Sync
            # dispatch, so the critical-path ones come first) ----
            nc.sync.dma_start(out=wk_sb, in_=wk_d)
            for t in range(DT):
                nc.sync.dma_start(out=xk_sb[:, t, :], in_=xk_d[t])
            nc.sync.dma_start(out=qkb_sb, in_=qkb_d)
            nc.sync.dma_start(out=kbias_sb, in_=kbias_d)
            nc.sync.dma_start(out=wq_sb, in_=wq_d)
            for t in range(DT):
                nc.sync.dma_start(out=xq_sb[:, t, :], in_=xq_d[t])
            nc.sync.dma_start(out=wv_sb, in_=wv_d)
            xv_src = bass.AP(tensor=xv_d.tensor, offset=xv_d.offset,
                             ap=[[SKP, 128], [128 * SKP, DT], [1, SKP]])
            nc.sync.dma_start(out=xv_sb, in_=xv_src)
            nc.sync.dma_start(out=vb_bc, in_=bass.AP(
                tensor=vb_d.tensor, offset=vb_d.offset,
                ap=[[0, 128], vb_d.ap[1]]))
            nc.sync.dma_start(out=wo_sb[:, 0, :], in_=wo_d[0])
            nc.sync.dma_start(out=wo_sb[:, 1, :], in_=wo_d[1])

            # ---- stage A: first K chunk + full Q projection, t-outer so
            # matmuls stream behind the per-d-tile DMAs; later K chunks are
            # emitted as fillers inside the first attention pass ----
            with tc.tile_pool(name="psA", bufs=3, space="PSUM") as psA:
                for w_sb, x_sb, pt_sb, blks, bcol0 in (
                        (wk_sb, xk_sb, kt_sb, kblk[:1], 2),
                        (wq_sb, xq_sb, qt_sb, qblk, 0)):
                    pp = [psA.tile([128, 2, 512], F32, tag="s",
                                   name=f"pp{ci}") for ci in range(len(blks))]
                    for t in range(DT):
                        b0 = 0
                        for ci, blen in enumerate(blks):
                            for p in range(2):
                                nc.tensor.matmul(
                                    pp[ci][:, p, :blen],
                                    w_sb[:, t, p * 128:(p + 1) * 128],
                                    x_sb[:, t, b0:b0 + blen],
                                    start=(t == 0), stop=(t == DT - 1))
                            b0 += blen
                    b0 = 0
                    for ci, blen in enumerate(blks):
                        for p in range(2):
                            nc.vector.tensor_scalar_add(
                                pt_sb[:, p, b0:b0 + blen], pp[ci][:, p, :blen],
                                qkb_sb[:, bcol0 + p: bcol0 + p + 1])
                        b0 += blen

            # ---- stage B/C ----
            with tc.tile_pool(name="psS", bufs=2, space="PSUM") as psS, \
                 tc.tile_pool(name="psO", bufs=2, space="PSUM") as psO:

                def vproj(skt):
                    pv = psS.tile([128, 2, 512], F32, tag="s", name="pv")
                    for t in range(DT):
                        nc.tensor.matmul(
                            pv[:, 0, :256],
                            xv_sb[:, t, skt * 128:(skt + 1) * 128],
                            wv_sb[:, t, :],
                            start=(t == 0), stop=(t == DT - 1))
                    vout = bass.AP(tensor=v_sb.tensor,
                                   offset=v_sb.offset + skt * 4 * 80,
                                   ap=[v_sb.ap[0], [80, 4], [1, 64]])
                    pv4 = bass.AP(tensor=pv.tensor, offset=pv.offset,
                                  ap=[pv.ap[0], [64, 4], [1, 64]])
                    vb4 = bass.AP(tensor=vb_bc.tensor, offset=vb_bc.offset,
                                  ap=[vb_bc.ap[0], [64, 4], [1, 64]])
                    nc.vector.tensor_add(vout, pv4, vb4)

                def scores_exp(bq0, W, pair, kt, epool, ebufs):
                    st = psS.tile([128, 2, 512], F32, tag="s")
                    for hh in range(2):
                        nc.tensor.matmul(
                            st[:, hh, :W],
                            kt_sb[hh * 64:(hh + 1) * 64, pair,
                                  kt * 128:(kt + 1) * 128],
                            qt_sb[hh * 64:(hh + 1) * 64, pair, bq0:bq0 + W],
                            start=True, stop=True)
                    e_t = epool.tile([128, 2, 512], F16, tag="e",
                                     bufs=ebufs)
                    nc.scalar.activation(
                        e_t[:, :, :W], st[:, :, :W],
                        mybir.ActivationFunctionType.Exp,
                        bias=kbias_sb[:, kt:kt + 1], scale=SCALE)
                    return e_t

                def av(op_t, pair, kt, e_t, W):
                    for hh in range(2):
                        h = pair * 2 + hh
                        nc.tensor.matmul(
                            op_t[:65, hh, :W],
                            v_sb[:, kt, h * 80:h * 80 + 65],
                            e_t[:, hh, :W],
                            start=(kt == 0), stop=(kt == SKT - 1))

                def emit_pass(bq0, W, pair, op_t, fillers):
                    for kt in range(SKT):
                        e_t = scores_exp(bq0, W, pair, kt, etile, 4)
                        if kt < len(fillers) and fillers[kt] is not None:
                            fillers[kt]()
                        av(op_t, pair, kt, e_t, W)
                    for f in fillers[SKT:]:
                        if f is not None:
                            f()

                def emit_norm(bq0, W, pair, op_t):
                    # one fast reciprocal covers both heads' z rows
                    zinv = work.tile([128, 2, 512], F32, tag="zinv")
                    nc.vector.reciprocal_approx_fast(
                        zinv[:, :, :W], op_t[:, :, :W])
                    zinv16 = work.tile([128, 2, 512], F16, tag="zinv16")
                    nc.vector.tensor_copy(zinv16[64:65, :, :W],
                                          zinv[64:65, :, :W])
                    for hh in range(2):
                        zbc = psS.tile([64, 512], F32, tag="s",
                                       name=f"zbc{hh}")
                        nc.tensor.matmul(
                            zbc[:, :W],
                            ones_h[64:65, 0:64],
                            zinv16[64:65, hh, :W],
                            start=True, stop=True,
                            tile_position=(64, 0))
                        zbc_sb = work.tile([64, 512], F32, tag="zbc",
                                           name=f"zbcs{hh}")
                        nc.vector.tensor_copy(zbc_sb[:, :W], zbc[:, :W])
                        nc.vector.tensor_mul(
                            ot_sb[hh * 64:(hh + 1) * 64, pair,
                                  bq0:bq0 + W],
                            op_t[0:64, hh, :W], zbc_sb[:, :W])

                def outproj_unit(q0, qw, on_scalar):
                    def emit():
                        po = psS.tile([128, 2, 512], F32, tag="s", name="po")
                        for ch in range(2):
                            for kt in range(2):
                                nc.tensor.matmul(
                                    po[:qw, ch, :],
                                    ot_sb[:, kt, q0:q0 + qw],
                                    wo_sb[:, kt, ch * 512:(ch + 1) * 512],
                                    start=(kt == 0), stop=(kt == 1))
                        ob = work.tile([128, OUT], F16, tag="ob")
                        pof = bass.AP(tensor=po.tensor, offset=po.offset,
                                      ap=[[po.ap[0][0], qw], [1, OUT]])
                        if on_scalar:
                            nc.scalar.copy(ob[:qw, :], pof)
                        else:
                            nc.vector.tensor_copy(ob[:qw, :], pof)
                        nc.sync.dma_start(out=outp[q0:q0 + qw, :],
                                          in_=ob[:qw, :])
                    return emit

                def outproj_units(bq0, W):
                    units = []
                    q0 = bq0
                    while q0 < bq0 + W:
                        qw = min(128, bq0 + W - q0)
                        units.append(outproj_unit(q0, qw, len(units) % 2))
                        q0 += qw
                    return units

                # late K-proj chunks: per-d-tile pieces accumulating into a
                # psO buffer (psO is otherwise idle until the first AV)
                def kproj_pieces(kpp, b0, blen):
                    pieces = []
                    for t in range(DT):
                        def piece(t=t):
                            for p in range(2):
                                nc.tensor.matmul(
                                    kpp[:, p, :blen],
                                    wk_sb[:, t, p * 128:(p + 1) * 128],
                                    xk_sb[:, t, b0:b0 + blen],
                                    start=(t == 0), stop=(t == DT - 1))
                        pieces.append(piece)
                    def bias():
                        for p in range(2):
                            nc.vector.tensor_scalar_add(
                                kt_sb[:, p, b0:b0 + blen], kpp[:, p, :blen],
                                qkb_sb[:, 2 + p:3 + p])
                    pieces.append(bias)
                    return pieces

                # --- pass (0,0): scores+exp only; its AV runs in pass (0,1)
                # together with the streaming V projection. Late K chunks
                # fill the idle PE here, 3 pieces per kt so chunk ci lands
                # before the first score tile that reads it ---
                W0 = qblk[0]
                e0 = []
                kfill = []
                koff = kblk[0]
                for ci, blen in enumerate(kblk[1:]):
                    kpp = psO.tile([128, 2, 512], F32, tag="acc",
                                   name=f"kpp{ci}")
                    kfill.extend(kproj_pieces(kpp, koff, blen))
                    koff += blen
                for kt in range(SKT):
                    e0.append(scores_exp(0, W0, 0, kt, e0tile, SKT))
                    for f in kfill[3 * kt:3 * kt + 3]:
                        f()
                for f in kfill[3 * SKT:]:
                    f()

                op00 = psO.tile([128, 2, 512], F32, tag="acc", name="op00")
                op01 = psO.tile([128, 2, 512], F32, tag="acc", name="op01")
                vproj(0)
                if SKT > 1:
                    vproj(1)

                def pass01_filler(kt):
                    def emit():
                        if kt + 2 < SKT:
                            vproj(kt + 2)
                        av(op00, 0, kt, e0[kt], W0)
                    return emit

                fillers = [pass01_filler(kt) for kt in range(SKT)]
                fillers.append(lambda: emit_norm(0, W0, 0, op00))
                emit_pass(0, W0, 1, op01, fillers)

                pend_norm = (0, W0, 1, op01)
                pend_out = (0, W0)
                bq0 = W0
                for bi in range(1, len(qblk)):
                    W = qblk[bi]
                    for pair in range(2):
                        fillers = [None]
                        if pend_norm is not None:
                            fillers.append(
                                (lambda a: lambda: emit_norm(*a))(pend_norm))
                            pend_norm = None
                        if pair == 0 and pend_out is not None:
                            fillers.extend([None, None])
                            fillers.extend(outproj_units(*pend_out))
                            pend_out = None
                        op_t = psO.tile([128, 2, 512], F32, tag="acc")
                        emit_pass(bq0, W, pair, op_t, fillers)
                        pend_norm = (bq0, W, pair, op_t)
                    pend_out = (bq0, W)
                    bq0 += W
                emit_norm(*pend_norm)
                for u in outproj_units(*pend_out):
                    u()

    nc.compile()
    return nc


_NC_CACHE = {}


def _get_kernel(QR, SKP):
    key = (QR, SKP)
    if key not in _NC_CACHE:
        _NC_CACHE[key] = build_kernel(QR, SKP)
    return _NC_CACHE[key]


def _ref_numpy(q, k, v, Wq, bq, Wk, bk, Wv, bv, Wo, bo, qm, vm):
    """Exact-reference fallback for degenerate masks (all-zero v_mask)."""
    qp = (q @ Wq + bq).reshape(S, H, HS)
    kp = (k @ Wk + bk).reshape(S, H, HS)
    vp = (v @ Wv + bv).reshape(S, H, HS)
    a = np.einsum('qhd,khd->hqk', qp, kp) / np.sqrt(HS)
    a = a - (1.0 - vm[None, None, :]) * 1e12
    a = a - a.max(-1, keepdims=True)
    e = np.exp(a)
    p = e / e.sum(-1, keepdims=True)
    o = np.einsum('hqk,khd->qhd', p, vp).reshape(S, H * HS)
    return (o @ Wo + bo) * qm[:, None]


def run(query, key, value, Wq, bq, Wk, bk, Wv, bv, Wo, bo, q_mask, v_mask,
        trace=False):
    query = np.asarray(query, np.float32)
    key = np.asarray(key, np.float32)
    value = np.asarray(value, np.float32)
    Wq, bq = np.asarray(Wq, np.float32), np.asarray(bq, np.float32)
    Wk, bk = np.asarray(Wk, np.float32), np.asarray(bk, np.float32)
    Wv, bv = np.asarray(Wv, np.float32), np.asarray(bv, np.float32)
    Wo, bo = np.asarray(Wo, np.float32), np.asarray(bo, np.float32)
    q_mask = np.asarray(q_mask)
    v_mask = np.asarray(v_mask)

    qidx = [np.nonzero(q_mask[b])[0] for b in range(B)]
    kidx = [np.nonzero(v_mask[b])[0] for b in range(B)]
    host_fallback = [len(kidx[b]) == 0 for b in range(B)]

    QR = max([128] + [len(i) for b, i in enumerate(qidx) if not host_fallback[b]])
    # small leftover beyond a 512 multiple goes to the host so the device
    # runs uniform 512-wide blocks
    if QR > 512 and QR % 512 <= 64:
        QR = 512 * (QR // 512)
    nk = max([128] + [len(i) for b, i in enumerate(kidx) if not host_fallback[b]])
    SKP = ((nk + 127) // 128) * 128
    SKT = SKP // 128
    QRP = ((QR + 31) // 32) * 32

    nc = _get_kernel(QR, SKP)

    in_maps = []
    for c in range(NCORES):
        b, hg = c // 4, c % 4
        hc = slice(hg * HPC * HS, (hg + 1) * HPC * HS)  # this core's 256 head cols
        xq = np.zeros((QRP, D), np.float32)
        xk = np.zeros((SKP, D), np.float32)
        xv = np.zeros((SKP, D), np.float32)
        if not host_fallback[b]:
            nq_dev = min(len(qidx[b]), QR)
            xq[:nq_dev] = query[b][qidx[b][:nq_dev]]
            xk[:len(kidx[b])] = key[b][kidx[b]]
            xv[:len(kidx[b])] = value[b][kidx[b]]
        qkb = np.stack([bq[hc][:128], bq[hc][128:],
                        bk[hc][:128], bk[hc][128:]], axis=1)
        nkb = len(kidx[b]) if not host_fallback[b] else 0
        kbias = np.where(np.arange(SKP) < nkb, 0.0, KPAD_BIAS).astype(np.float32)
        in_maps.append({
            'xq': np.ascontiguousarray(xq.T.reshape(DT, 128, QRP)).astype(np.float16),
            'xk': np.ascontiguousarray(xk.T.reshape(DT, 128, SKP)).astype(np.float16),
            'xv': np.ascontiguousarray(xv.T.reshape(DT, 128, SKP)).astype(np.float16),
            'wq': np.ascontiguousarray(Wq[:, hc].reshape(DT, 128, 256).transpose(1, 0, 2)).astype(np.float16),
            'wk': np.ascontiguousarray(Wk[:, hc].reshape(DT, 128, 256).transpose(1, 0, 2)).astype(np.float16),
            'wv': np.ascontiguousarray(Wv[:, hc].reshape(DT, 128, 256).transpose(1, 0, 2)).astype(np.float16),
            'wo': np.ascontiguousarray(Wo[hc, :].reshape(2, 128, OUT)).astype(np.float16),
            'qkb': np.ascontiguousarray(qkb),
            'vb': np.ascontiguousarray(bv[hc].reshape(1, 256)),
            'kbias': np.ascontiguousarray(kbias.reshape(SKT, 128).T),
        })

    res = bass_utils.run_bass_kernel_spmd(
        nc, in_maps, core_ids=list(range(NCORES)), trace=trace)

    out = np.zeros((B, S, OUT), np.float32)
    for b in range(B):
        if host_fallback[b]:
            out[b] = _ref_numpy(query[b], key[b], value[b], Wq, bq, Wk, bk,
                                Wv, bv, Wo, bo,
                                q_mask[b].astype(np.float32),
                                v_mask[b].astype(np.float32))
            continue
        acc = np.zeros((QR, OUT), np.float32)
        for c in range(4 * b, 4 * b + 4):
            acc += res.results[c]['outp'].astype(np.float32)
        nqb = len(qidx[b])
        nq_dev = min(nqb, QR)
        out[b][qidx[b][:nq_dev]] = acc[:nq_dev] + bo
        if nqb > nq_dev:
            # host-assist for the few query rows beyond the 512-multiple
            rows = qidx[b][nq_dev:]
            qp = (query[b][rows] @ Wq + bq).reshape(-1, H, HS)
            kp = (key[b][kidx[b]] @ Wk + bk).reshape(-1, H, HS)
            vp = (value[b][kidx[b]] @ Wv + bv).reshape(-1, H, HS)
            a = np.einsum('qhd,khd->hqk', qp, kp) / np.sqrt(HS)
            a = a - a.max(-1, keepdims=True)
            e = np.exp(a)
            p = e / e.sum(-1, keepdims=True)
            o = np.einsum('hqk,khd->qhd', p, vp).reshape(len(rows), H * HS)
            out[b][rows] = o @ Wo + bo
    return out, res


def kernel(**inputs):
    out, _ = run(**inputs)
    return out
